# revision 7
# baseline (speedup 1.0000x reference)
"""Trainium2 Bass kernel for nn_DMGAGRUcell (GRU cell with graph-conv gates).

Math (per batch b):
  x    = [inputs | hx]                      (N, 66)
  x1   = S @ x, x2 = adp[b] @ x             (diffusion + adaptive hop)
  ru   = sigmoid([x|x1|x2]_interleaved @ W_ru);  r, u = split(ru)
  c    = tanh([x|x1|x2']_interleaved @ W_c)  with x' = [inputs | r*hx]
  out  = u*hx + (1-u)*c

Sharding: 2 batches per core x 8 cores (data parallel over B=16).
Device layout is feature-major (transposed): all gconv outputs are computed
as x1T = x.T @ S.T etc. with the small x as the PE stationary operand and the
big matrix streaming; adp[b] (bf16, host-pre-transposed) stays resident in
SBUF so HBM reads it once per batch. The dominant-magnitude gate chunks
(x0 @ W0, rh @ W) run in fp32; the small x1/x2 chunks run in bf16.
"""

import os
import numpy as np
import ml_dtypes

BF16 = ml_dtypes.bfloat16

N = 2048
B = 16
D_IN = 2
UNITS = 64
F = 66
B_LOC = 2          # batches per core
N_CORES = 8
KC = 16            # k chunks of 128 nodes
NS = 4             # 512-wide output slabs

_CACHE = {}


def _build():
    if "nc" in _CACHE:
        return _CACHE["nc"]

    from contextlib import ExitStack
    import concourse.mybir as mybir
    import concourse.tile as tile
    from concourse import bacc

    f32 = mybir.dt.float32
    bf = mybir.dt.bfloat16
    AF = mybir.ActivationFunctionType

    nc = bacc.Bacc("TRN2", target_bir_lowering=False, debug=False,
                   num_devices=N_CORES)

    adpT_d = nc.dram_tensor("adpT", [B_LOC, KC, 128, N], bf, kind="ExternalInput")
    sT_d = nc.dram_tensor("sT", [KC, 128, N], bf, kind="ExternalInput")
    xnm_d = nc.dram_tensor("xnm", [B_LOC, 128, KC, F], bf, kind="ExternalInput")
    x0Tf_d = nc.dram_tensor("x0Tf", [B_LOC, F, N], f32, kind="ExternalInput")
    hxTf_d = nc.dram_tensor("hxTf", [B_LOC, UNITS, N], f32, kind="ExternalInput")
    wru0_d = nc.dram_tensor("wru0f", [F, 128], f32, kind="ExternalInput")
    wru1_d = nc.dram_tensor("wru1b", [F, 128], bf, kind="ExternalInput")
    wru2_d = nc.dram_tensor("wru2b", [F, 128], bf, kind="ExternalInput")
    wcinp_d = nc.dram_tensor("wcinpf", [D_IN, UNITS], f32, kind="ExternalInput")
    wcrh_d = nc.dram_tensor("wcrhf", [UNITS, UNITS], f32, kind="ExternalInput")
    wc1_d = nc.dram_tensor("wc1b", [F, UNITS], bf, kind="ExternalInput")
    wc2_d = nc.dram_tensor("wc2b", [F, UNITS], bf, kind="ExternalInput")
    id_d = nc.dram_tensor("ident", [UNITS, UNITS], f32, kind="ExternalInput")
    out_d = nc.dram_tensor("outT", [B_LOC, UNITS, N], f32, kind="ExternalOutput")

    with tile.TileContext(nc) as tc, ExitStack() as ctx:
        spool = ctx.enter_context(tc.tile_pool(name="spool", bufs=1))
        apool = ctx.enter_context(tc.tile_pool(name="apool", bufs=16))
        cpool = ctx.enter_context(tc.tile_pool(name="cpool", bufs=1))
        wpool = ctx.enter_context(tc.tile_pool(name="wpool", bufs=1))
        w2pool = ctx.enter_context(tc.tile_pool(name="w2pool", bufs=2))
        pp = ctx.enter_context(tc.tile_pool(name="pp", bufs=6, space="PSUM"))
        pt = ctx.enter_context(tc.tile_pool(name="pt", bufs=2, space="PSUM"))

        s_tiles = []
        for k in range(KC):
            t = spool.tile([128, N], bf, tag=f"s{k}", name=f"s{k}")
            nc.sync.dma_start(t[:], sT_d[k])
            s_tiles.append(t)

        def const(name, dram, shape, dt):
            t = cpool.tile(shape, dt, tag=name, name=name)
            nc.sync.dma_start(t[:], dram[:])
            return t

        wru0 = const("wru0", wru0_d, [F, 128], f32)
        wru1 = const("wru1", wru1_d, [F, 128], bf)
        wru2 = const("wru2", wru2_d, [F, 128], bf)
        wcinp = const("wcinp", wcinp_d, [D_IN, UNITS], f32)
        wcrh = const("wcrh", wcrh_d, [UNITS, UNITS], f32)
        wc1 = const("wc1", wc1_d, [F, UNITS], bf)
        wc2 = const("wc2", wc2_d, [F, UNITS], bf)
        ident = const("ident", id_d, [UNITS, UNITS], f32)

        for b in range(B_LOC):
            xnm = w2pool.tile([128, KC, F], bf, tag="xnm")
            nc.sync.dma_start(xnm[:], xnm_d[b])
            x0Tf = wpool.tile([F, N], f32, tag="x0Tf")
            nc.sync.dma_start(x0Tf[:], x0Tf_d[b])
            hxTf = wpool.tile([UNITS, N], f32, tag="hxTf")
            nc.sync.dma_start(hxTf[:], hxTf_d[b])

            a_tiles = []
            for k in range(KC):
                t = apool.tile([128, N], bf, tag="adp", name=f"adp_{b}_{k}")
                nc.sync.dma_start(t[:], adpT_d[b, k])
                a_tiles.append(t)

            def gconv(x1_dst, x2_dst):
                # x1T = x.T @ S.T ; x2T = x.T @ adp[b].T   (66, 2048) each
                for rhs_tiles, dst in ((s_tiles, x1_dst), (a_tiles, x2_dst)):
                    ps = [pp.tile([F, 512], f32, tag="ps", name=f"ps_g{s}") for s in range(NS)]
                    for k in range(KC):
                        lhsT = xnm[:, k, :]
                        for s in range(NS):
                            nc.tensor.matmul(
                                ps[s][:], lhsT,
                                rhs_tiles[k][:, s * 512:(s + 1) * 512],
                                start=(k == 0), stop=(k == KC - 1))
                    for s in range(NS):
                        nc.vector.tensor_copy(
                            dst[:, s * 512:(s + 1) * 512], ps[s][:])

            # ---- gconv 1 ----
            x1T = w2pool.tile([F, N], bf, tag="x1T")
            x2T = w2pool.tile([F, N], bf, tag="x2T")
            gconv(x1T, x2T)

            # ru = sigmoid(x0.W0 + x1.W1 + x2.W2). r and u are computed as
            # separate accumulation groups (W free-dim split) so both land at
            # partitions 0-63 - two-input DVE ops need equal base partitions.
            ract = wpool.tile([UNITS, N], f32, tag="ract")
            uact = wpool.tile([UNITS, N], f32, tag="uact")
            for s in range(NS):
                sl = slice(s * 512, (s + 1) * 512)
                for g, (dst, wsl) in enumerate(
                        ((ract, slice(0, UNITS)), (uact, slice(UNITS, 128)))):
                    ps = pp.tile([UNITS, 512], f32, tag="ps", name=f"ps_ru{s}_{g}")
                    nc.tensor.matmul(ps[:], wru0[:, wsl], x0Tf[:, sl], start=True, stop=False)
                    nc.tensor.matmul(ps[:], wru1[:, wsl], x1T[:, sl], start=False, stop=False)
                    nc.tensor.matmul(ps[:], wru2[:, wsl], x2T[:, sl], start=False, stop=True)
                    nc.scalar.activation(dst[:, sl], ps[:], AF.Sigmoid)

            # rh = r * hx (fp32, partitions 0-63)
            rhf = wpool.tile([UNITS, N], f32, tag="rhf")
            nc.vector.tensor_mul(rhf[:], ract[:], hxTf[:])

            # transpose rh back to node-major into xnm cols 2:66
            for k in range(KC):
                pst = pt.tile([128, UNITS], f32, tag="pst", name=f"pst_{k}")
                nc.tensor.transpose(
                    pst[:], rhf[:, k * 128:(k + 1) * 128], ident[:])
                nc.vector.tensor_copy(xnm[:, k, D_IN:F], pst[:])

            # ---- gconv 2 ----
            x1p = w2pool.tile([F, N], bf, tag="x1T")
            x2p = w2pool.tile([F, N], bf, tag="x2T")
            gconv(x1p, x2p)

            # c = tanh(inp.Wc[0:2] + rh.Wc[2:66] + x1'.Wc1 + x2'.Wc2)
            cT = wpool.tile([UNITS, N], f32, tag="cT")
            outT = rhf  # reuse: rhf's last reader is the wc-mm rh chunk
            for s in range(NS):
                sl = slice(s * 512, (s + 1) * 512)
                ps = pp.tile([UNITS, 512], f32, tag="ps", name=f"ps_c{s}")
                nc.tensor.matmul(ps[:], wcinp[:], x0Tf[0:D_IN, sl], start=True, stop=False)
                nc.tensor.matmul(ps[:], wcrh[:], rhf[:, sl], start=False, stop=False)
                nc.tensor.matmul(ps[:], wc1[:], x1p[:, sl], start=False, stop=False)
                nc.tensor.matmul(ps[:], wc2[:], x2p[:, sl], start=False, stop=True)
                nc.scalar.activation(cT[:, sl], ps[:], AF.Tanh)
                # out = c + u*(hx - c)
                nc.vector.tensor_sub(outT[:, sl], hxTf[:, sl], cT[:, sl])
                nc.vector.tensor_mul(outT[:, sl], uact[:, sl], outT[:, sl])
                nc.vector.tensor_add(outT[:, sl], outT[:, sl], cT[:, sl])
            nc.sync.dma_start(out_d[b], outT[:])

    nc.compile()
    _CACHE["nc"] = nc
    return nc


def _prep_host(inputs, hx, adp, support_rows, support_cols, support_vals,
               W_ru, W_c):
    xcat = np.concatenate(
        [inputs.reshape(B, N, D_IN), hx.reshape(B, N, UNITS)], axis=2)
    xcat = np.ascontiguousarray(xcat, dtype=np.float32)

    S = np.zeros((N, N), np.float32)
    np.add.at(S, (support_rows, support_cols), support_vals)
    sT = np.ascontiguousarray(S.T).astype(BF16).reshape(KC, 128, N)

    adp_bf = adp.astype(BF16)
    adpT = np.ascontiguousarray(adp_bf.transpose(0, 2, 1)).reshape(B, KC, 128, N)

    xnm = xcat.astype(BF16).reshape(B, KC, 128, F).transpose(0, 2, 1, 3)
    xnm = np.ascontiguousarray(xnm)
    x0Tf = np.ascontiguousarray(xcat.transpose(0, 2, 1))
    hxTf = np.ascontiguousarray(x0Tf[:, D_IN:F])

    wru = {
        "wru0f": np.ascontiguousarray(W_ru[0::3]).astype(np.float32),
        "wru1b": np.ascontiguousarray(W_ru[1::3]).astype(BF16),
        "wru2b": np.ascontiguousarray(W_ru[2::3]).astype(BF16),
    }
    wc0 = np.ascontiguousarray(W_c[0::3])
    wcd = {
        "wcinpf": np.ascontiguousarray(wc0[0:D_IN]).astype(np.float32),
        "wcrhf": np.ascontiguousarray(wc0[D_IN:F]).astype(np.float32),
        "wc1b": np.ascontiguousarray(W_c[1::3]).astype(BF16),
        "wc2b": np.ascontiguousarray(W_c[2::3]).astype(BF16),
    }
    ident = np.eye(UNITS, dtype=np.float32)

    shared = {"sT": sT, "ident": ident, **wru, **wcd}
    in_maps = []
    for c in range(N_CORES):
        lo, hi = c * B_LOC, (c + 1) * B_LOC
        in_maps.append({
            "adpT": np.ascontiguousarray(adpT[lo:hi]),
            "xnm": np.ascontiguousarray(xnm[lo:hi]),
            "x0Tf": np.ascontiguousarray(x0Tf[lo:hi]),
            "hxTf": np.ascontiguousarray(hxTf[lo:hi]),
            **shared,
        })
    return in_maps


def kernel(inputs, hx, adp, support_rows, support_cols, support_vals,
           W_ru, W_c, time_axis=None):
    from concourse.bass_utils import run_bass_kernel_spmd

    inputs = np.asarray(inputs, dtype=np.float32)
    hx = np.asarray(hx, dtype=np.float32)
    adp = np.asarray(adp, dtype=np.float32)
    support_rows = np.asarray(support_rows)
    support_cols = np.asarray(support_cols)
    support_vals = np.asarray(support_vals, dtype=np.float32)
    W_ru = np.asarray(W_ru, dtype=np.float32)
    W_c = np.asarray(W_c, dtype=np.float32)

    nc = _build()
    in_maps = _prep_host(inputs, hx, adp, support_rows, support_cols,
                         support_vals, W_ru, W_c)

    res = run_bass_kernel_spmd(nc, in_maps, core_ids=list(range(N_CORES)),
                               trace=False)
    _CACHE["last_result"] = res

    out = np.empty((B, N * UNITS), np.float32)
    for c in range(N_CORES):
        outT = res.results[c]["outT"]  # (B_LOC, 64, N)
        for i in range(B_LOC):
            out[c * B_LOC + i] = np.ascontiguousarray(
                outT[i].T).reshape(N * UNITS)
    return out


# revision 13
# speedup vs baseline: 1.2781x; 1.2781x over previous
"""Trainium2 Bass kernel for nn_DMGAGRUcell (GRU cell with graph-conv gates).

Math (per batch b):
  x    = [inputs | hx]                      (N, 66)
  x1   = S @ x, x2 = adp[b] @ x             (diffusion + adaptive hop)
  ru   = sigmoid([x|x1|x2]_interleaved @ W_ru);  r, u = split(ru)
  c    = tanh([x|x1|x2']_interleaved @ W_c)  with x' = [inputs | r*hx]
  out  = u*hx + (1-u)*c

Sharding: 2 batches per core x 8 cores (data parallel over B=16).
Device layout is feature-major (transposed): all gconv outputs are computed
as x1T = x.T @ S.T etc. with the small x as the PE stationary operand and the
big matrix streaming; adp[b] (bf16, host-pre-transposed) stays resident in
SBUF so HBM reads it once per batch. The dominant-magnitude gate chunks
(x0 @ W0, rh @ W) run in fp32; the small x1/x2 chunks run in bf16.
"""

import os
import numpy as np
import ml_dtypes

BF16 = ml_dtypes.bfloat16

N = 2048
B = 16
D_IN = 2
UNITS = 64
F = 66
B_LOC = 2          # batches per core
N_CORES = 8
KC = 16            # k chunks of 128 nodes
NS = 4             # 512-wide output slabs

_CACHE = {}


def _build():
    if "nc" in _CACHE:
        return _CACHE["nc"]

    from contextlib import ExitStack
    import concourse.mybir as mybir
    import concourse.tile as tile
    from concourse import bacc

    f32 = mybir.dt.float32
    bf = mybir.dt.bfloat16
    AF = mybir.ActivationFunctionType

    nc = bacc.Bacc("TRN2", target_bir_lowering=False, debug=False,
                   num_devices=N_CORES)

    adpT_d = nc.dram_tensor("adpT", [B_LOC, KC, 128, N], bf, kind="ExternalInput")
    sT_d = nc.dram_tensor("sT", [KC, 128, N], bf, kind="ExternalInput")
    xnm_d = nc.dram_tensor("xnm", [B_LOC, 128, KC, F], bf, kind="ExternalInput")
    x0Tb_d = nc.dram_tensor("x0Tb", [B_LOC, F, N], bf, kind="ExternalInput")
    hxTf_d = nc.dram_tensor("hxTf", [B_LOC, UNITS, N], f32, kind="ExternalInput")
    wru0_d = nc.dram_tensor("wru0b", [F, 128], bf, kind="ExternalInput")
    wru1_d = nc.dram_tensor("wru1b", [F, 128], bf, kind="ExternalInput")
    wru2_d = nc.dram_tensor("wru2b", [F, 128], bf, kind="ExternalInput")
    wcinp_d = nc.dram_tensor("wcinpb", [D_IN, UNITS], bf, kind="ExternalInput")
    wcrh_d = nc.dram_tensor("wcrhb", [UNITS, UNITS], bf, kind="ExternalInput")
    wc1_d = nc.dram_tensor("wc1b", [F, UNITS], bf, kind="ExternalInput")
    wc2_d = nc.dram_tensor("wc2b", [F, UNITS], bf, kind="ExternalInput")
    id_d = nc.dram_tensor("ident", [UNITS, UNITS], bf, kind="ExternalInput")
    out_d = nc.dram_tensor("outT", [B_LOC, UNITS, N], f32, kind="ExternalOutput")

    with tile.TileContext(nc) as tc, ExitStack() as ctx:
        spool = ctx.enter_context(tc.tile_pool(name="spool", bufs=1))
        apool = ctx.enter_context(tc.tile_pool(name="apool", bufs=16))
        cpool = ctx.enter_context(tc.tile_pool(name="cpool", bufs=1))
        wpool = ctx.enter_context(tc.tile_pool(name="wpool", bufs=1))
        w2pool = ctx.enter_context(tc.tile_pool(name="w2pool", bufs=2))
        pp = ctx.enter_context(tc.tile_pool(name="pp", bufs=8, space="PSUM"))

        # both batches' matmul inputs first so the PE can start early and
        # the fused gconv1 S-pass has both xnm tiles; hx (elementwise-only,
        # needed late) loads after the S stream
        binp = {}
        for b in range(B_LOC):
            xnm = w2pool.tile([128, KC, F], bf, tag="xnm", name=f"xnm{b}")
            nc.sync.dma_start(xnm[:], xnm_d[b])
            x0Tb = w2pool.tile([F, N], bf, tag="x0Tb", name=f"x0Tb{b}")
            nc.sync.dma_start(x0Tb[:], x0Tb_d[b])
            binp[b] = [xnm, x0Tb]

        s_tiles = []
        for k in range(KC):
            t = spool.tile([128, N], bf, tag=f"s{k}", name=f"s{k}")
            nc.sync.dma_start(t[:], sT_d[k])
            s_tiles.append(t)

        for b in range(B_LOC):
            hxTf = wpool.tile([UNITS, N], f32, tag="hxTf", name=f"hxTf{b}")
            nc.sync.dma_start(hxTf[:], hxTf_d[b])
            binp[b].append(hxTf)

        def const(name, dram, shape, dt):
            t = cpool.tile(shape, dt, tag=name, name=name)
            nc.sync.dma_start(t[:], dram[:])
            return t

        wru0 = const("wru0", wru0_d, [F, 128], bf)
        wru1 = const("wru1", wru1_d, [F, 128], bf)
        wru2 = const("wru2", wru2_d, [F, 128], bf)
        wcinp = const("wcinp", wcinp_d, [D_IN, UNITS], bf)
        wcrh = const("wcrh", wcrh_d, [UNITS, UNITS], bf)
        wc1 = const("wc1", wc1_d, [F, UNITS], bf)
        wc2 = const("wc2", wc2_d, [F, UNITS], bf)
        ident = const("ident", id_d, [UNITS, UNITS], bf)

        def stream_pass(lhs_xnms, rhs_tiles, dsts, drain_act, pfx):
            # dsts[i] = lhs_xnms[i].T @ rhs_tiles.T, k-major so several
            # batches' matmuls interleave behind one streamed rhs
            nb = len(lhs_xnms)
            ps = [[pp.tile([F, 512], f32, tag="ps", name=f"ps_{pfx}_{i}_{s}")
                   for s in range(NS)] for i in range(nb)]
            for k in range(KC):
                for i in range(nb):
                    lhsT = lhs_xnms[i][:, k, :]
                    for s in range(NS):
                        nc.tensor.matmul(
                            ps[i][s][:], lhsT,
                            rhs_tiles[k][:, s * 512:(s + 1) * 512],
                            start=(k == 0), stop=(k == KC - 1))
            # alternate drain engines so consumers are not paced by a
            # single engine's serialized copies
            for i in range(nb):
                for s in range(NS):
                    dsl = dsts[i][:, s * 512:(s + 1) * 512]
                    if (s + i) % 2 == (0 if drain_act else 1):
                        nc.scalar.activation(dsl, ps[i][s][:], AF.Copy)
                    else:
                        nc.vector.tensor_copy(dsl, ps[i][s][:])

        # ---- gconv 1 S-passes, both batches fused behind one S stream ----
        x1Ts = [w2pool.tile([F, N], bf, tag="x1T", name=f"x1T{b}")
                for b in range(B_LOC)]
        stream_pass([binp[0][0], binp[1][0]], s_tiles, x1Ts, True, "s1")

        for b in range(B_LOC):
            xnm, x0Tb, hxTf = binp[b]

            a_tiles = []
            for k in range(KC):
                t = apool.tile([128, N], bf, tag="adp", name=f"adp_{b}_{k}")
                nc.sync.dma_start(t[:], adpT_d[b, k])
                a_tiles.append(t)

            # ---- gconv 1 adp-pass ----
            x1T = x1Ts[b]
            x2T = w2pool.tile([F, N], bf, tag="x2T")
            stream_pass([xnm], a_tiles, [x2T], False, f"a1_{b}")

            # ru = sigmoid(x0.W0 + x1.W1 + x2.W2). r and u are computed as
            # separate accumulation groups (W free-dim split) so both land at
            # partitions 0-63 - two-input DVE ops need equal base partitions.
            # ru = sigmoid([x0|x1|x2] @ W_ru): one M=128 accumulation group
            # per slab; r (rows 0-63) and u (rows 64-127) drain via separate
            # sigmoids, u with a shifted partition base down to 0-63.
            # rh = r*hx follows per slab on the DVE; the PE transposes of rh
            # run after all ru matmuls so their input chain is already done.
            ract = wpool.tile([UNITS, N], f32, tag="ract")
            uact = wpool.tile([UNITS, N], f32, tag="uact")
            rhb = wpool.tile([UNITS, N], bf, tag="rhb")
            for s in range(NS):
                sl = slice(s * 512, (s + 1) * 512)
                ps = pp.tile([128, 512], f32, tag="ps", name=f"ps_ru{s}")
                nc.tensor.matmul(ps[:], wru0[:], x0Tb[:, sl], start=True, stop=False)
                nc.tensor.matmul(ps[:], wru1[:], x1T[:, sl], start=False, stop=False)
                nc.tensor.matmul(ps[:], wru2[:], x2T[:, sl], start=False, stop=True)
                nc.scalar.activation(ract[:, sl], ps[0:UNITS, :], AF.Sigmoid)
                nc.scalar.activation(uact[:, sl], ps[UNITS:128, :], AF.Sigmoid)
                nc.vector.tensor_mul(rhb[:, sl], ract[:, sl], hxTf[:, sl])
            for k in range(KC):
                pst = pp.tile([128, 1024], bf, tag="ps", name=f"pst_{k}")
                nc.tensor.transpose(
                    pst[:, 0:UNITS], rhb[:, k * 128:(k + 1) * 128], ident[:])
                nc.vector.tensor_copy(xnm[:, k, D_IN:F], pst[:, 0:UNITS])

            # ---- gconv 2 ----
            x1p = w2pool.tile([F, N], bf, tag="x1T")
            x2p = w2pool.tile([F, N], bf, tag="x2T")
            stream_pass([xnm], s_tiles, [x1p], True, f"s2_{b}")
            stream_pass([xnm], a_tiles, [x2p], False, f"a2_{b}")

            # c = tanh(inp.Wc[0:2] + rh.Wc[2:66] + x1'.Wc1 + x2'.Wc2)
            cT = wpool.tile([UNITS, N], f32, tag="cT")
            outT = wpool.tile([UNITS, N], f32, tag="outT")
            for s in range(NS):
                sl = slice(s * 512, (s + 1) * 512)
                ps = pp.tile([UNITS, 512], f32, tag="ps", name=f"ps_c{s}")
                nc.tensor.matmul(ps[:], wcinp[:], x0Tb[0:D_IN, sl], start=True, stop=False)
                nc.tensor.matmul(ps[:], wcrh[:], rhb[:, sl], start=False, stop=False)
                nc.tensor.matmul(ps[:], wc1[:], x1p[:, sl], start=False, stop=False)
                nc.tensor.matmul(ps[:], wc2[:], x2p[:, sl], start=False, stop=True)
                nc.scalar.activation(cT[:, sl], ps[:], AF.Tanh)
                # out = c + u*(hx - c)
                nc.vector.tensor_sub(outT[:, sl], hxTf[:, sl], cT[:, sl])
                nc.vector.tensor_mul(outT[:, sl], uact[:, sl], outT[:, sl])
                nc.vector.tensor_add(outT[:, sl], outT[:, sl], cT[:, sl])
            nc.sync.dma_start(out_d[b], outT[:])

    nc.compile()
    _CACHE["nc"] = nc
    return nc


def _prep_host(inputs, hx, adp, support_rows, support_cols, support_vals,
               W_ru, W_c):
    xcat = np.concatenate(
        [inputs.reshape(B, N, D_IN), hx.reshape(B, N, UNITS)], axis=2)
    xcat = np.ascontiguousarray(xcat, dtype=np.float32)

    S = np.zeros((N, N), np.float32)
    np.add.at(S, (support_rows, support_cols), support_vals)
    sT = np.ascontiguousarray(S.T).astype(BF16).reshape(KC, 128, N)

    adp_bf = adp.astype(BF16)
    adpT = np.ascontiguousarray(adp_bf.transpose(0, 2, 1)).reshape(B, KC, 128, N)

    xnm = xcat.astype(BF16).reshape(B, KC, 128, F).transpose(0, 2, 1, 3)
    xnm = np.ascontiguousarray(xnm)
    x0T = np.ascontiguousarray(xcat.transpose(0, 2, 1))
    x0Tb = x0T.astype(BF16)
    hxTf = np.ascontiguousarray(x0T[:, D_IN:F])

    wru = {
        "wru0b": np.ascontiguousarray(W_ru[0::3]).astype(BF16),
        "wru1b": np.ascontiguousarray(W_ru[1::3]).astype(BF16),
        "wru2b": np.ascontiguousarray(W_ru[2::3]).astype(BF16),
    }
    wc0 = np.ascontiguousarray(W_c[0::3])
    wcd = {
        "wcinpb": np.ascontiguousarray(wc0[0:D_IN]).astype(BF16),
        "wcrhb": np.ascontiguousarray(wc0[D_IN:F]).astype(BF16),
        "wc1b": np.ascontiguousarray(W_c[1::3]).astype(BF16),
        "wc2b": np.ascontiguousarray(W_c[2::3]).astype(BF16),
    }
    ident = np.eye(UNITS, dtype=BF16)

    shared = {"sT": sT, "ident": ident, **wru, **wcd}
    in_maps = []
    for c in range(N_CORES):
        lo, hi = c * B_LOC, (c + 1) * B_LOC
        in_maps.append({
            "adpT": np.ascontiguousarray(adpT[lo:hi]),
            "xnm": np.ascontiguousarray(xnm[lo:hi]),
            "x0Tb": np.ascontiguousarray(x0Tb[lo:hi]),
            "hxTf": np.ascontiguousarray(hxTf[lo:hi]),
            **shared,
        })
    return in_maps


def kernel(inputs, hx, adp, support_rows, support_cols, support_vals,
           W_ru, W_c, time_axis=None):
    from concourse.bass_utils import run_bass_kernel_spmd

    inputs = np.asarray(inputs, dtype=np.float32)
    hx = np.asarray(hx, dtype=np.float32)
    adp = np.asarray(adp, dtype=np.float32)
    support_rows = np.asarray(support_rows)
    support_cols = np.asarray(support_cols)
    support_vals = np.asarray(support_vals, dtype=np.float32)
    W_ru = np.asarray(W_ru, dtype=np.float32)
    W_c = np.asarray(W_c, dtype=np.float32)

    nc = _build()
    in_maps = _prep_host(inputs, hx, adp, support_rows, support_cols,
                         support_vals, W_ru, W_c)

    res = run_bass_kernel_spmd(nc, in_maps, core_ids=list(range(N_CORES)),
                               trace=False)
    _CACHE["last_result"] = res

    out = np.empty((B, N * UNITS), np.float32)
    for c in range(N_CORES):
        outT = res.results[c]["outT"]  # (B_LOC, 64, N)
        for i in range(B_LOC):
            out[c * B_LOC + i] = np.ascontiguousarray(
                outT[i].T).reshape(N * UNITS)
    return out


# revision 16
# speedup vs baseline: 1.2921x; 1.0110x over previous
"""Trainium2 Bass kernel for nn_DMGAGRUcell (GRU cell with graph-conv gates).

Math (per batch b):
  x    = [inputs | hx]                      (N, 66)
  x1   = S @ x, x2 = adp[b] @ x             (diffusion + adaptive hop)
  ru   = sigmoid([x|x1|x2]_interleaved @ W_ru);  r, u = split(ru)
  c    = tanh([x|x1|x2']_interleaved @ W_c)  with x' = [inputs | r*hx]
  out  = u*hx + (1-u)*c

Sharding: 2 batches per core x 8 cores (data parallel over B=16).
Device layout is feature-major (transposed): all gconv outputs are computed
as x1T = x.T @ S.T etc. with the small x as the PE stationary operand and the
big matrix streaming; adp[b] (bf16, host-pre-transposed) stays resident in
SBUF so HBM reads it once per batch. The dominant-magnitude gate chunks
(x0 @ W0, rh @ W) run in fp32; the small x1/x2 chunks run in bf16.
"""

import os
import numpy as np
import ml_dtypes

BF16 = ml_dtypes.bfloat16

N = 2048
B = 16
D_IN = 2
UNITS = 64
F = 66
B_LOC = 2          # batches per core
N_CORES = 8
KC = 16            # k chunks of 128 nodes
NS = 4             # 512-wide output slabs

_CACHE = {}


def _build():
    if "nc" in _CACHE:
        return _CACHE["nc"]

    from contextlib import ExitStack
    import concourse.mybir as mybir
    import concourse.tile as tile
    from concourse import bacc

    f32 = mybir.dt.float32
    bf = mybir.dt.bfloat16
    AF = mybir.ActivationFunctionType

    nc = bacc.Bacc("TRN2", target_bir_lowering=False, debug=False,
                   num_devices=N_CORES)

    adpT_d = nc.dram_tensor("adpT", [B_LOC, KC, 128, N], bf, kind="ExternalInput")
    sT_d = nc.dram_tensor("sT", [KC, 128, N], bf, kind="ExternalInput")
    xnm_d = nc.dram_tensor("xnm", [B_LOC, 128, KC, F], bf, kind="ExternalInput")
    x0Tb_d = nc.dram_tensor("x0Tb", [B_LOC, F, N], bf, kind="ExternalInput")
    hxTf_d = nc.dram_tensor("hxTf", [B_LOC, UNITS, N], f32, kind="ExternalInput")
    wru0_d = nc.dram_tensor("wru0b", [F, 128], bf, kind="ExternalInput")
    wru1_d = nc.dram_tensor("wru1b", [F, 128], bf, kind="ExternalInput")
    wru2_d = nc.dram_tensor("wru2b", [F, 128], bf, kind="ExternalInput")
    wcinp_d = nc.dram_tensor("wcinpb", [D_IN, UNITS], bf, kind="ExternalInput")
    wcrh_d = nc.dram_tensor("wcrhb", [UNITS, UNITS], bf, kind="ExternalInput")
    wc1_d = nc.dram_tensor("wc1b", [F, UNITS], bf, kind="ExternalInput")
    wc2_d = nc.dram_tensor("wc2b", [F, UNITS], bf, kind="ExternalInput")
    id_d = nc.dram_tensor("ident", [UNITS, UNITS], bf, kind="ExternalInput")
    out_d = nc.dram_tensor("outT", [B_LOC, UNITS, N], f32, kind="ExternalOutput")

    with tile.TileContext(nc) as tc, ExitStack() as ctx:
        spool = ctx.enter_context(tc.tile_pool(name="spool", bufs=1))
        apool = ctx.enter_context(tc.tile_pool(name="apool", bufs=16))
        cpool = ctx.enter_context(tc.tile_pool(name="cpool", bufs=1))
        wpool = ctx.enter_context(tc.tile_pool(name="wpool", bufs=1))
        w2pool = ctx.enter_context(tc.tile_pool(name="w2pool", bufs=2))
        pp = ctx.enter_context(tc.tile_pool(name="pp", bufs=8, space="PSUM"))

        # DMA order tracks first use: xnm0, s0 (the first matmuls' inputs),
        # xnm1, the rest of the S stream, then the late-needed inputs
        binp = {}
        xnm0 = w2pool.tile([128, KC, F], bf, tag="xnm", name="xnm0")
        nc.sync.dma_start(xnm0[:], xnm_d[0])
        s_tiles = [spool.tile([128, N], bf, tag="s0", name="s0")]
        nc.sync.dma_start(s_tiles[0][:], sT_d[0])
        xnm1 = w2pool.tile([128, KC, F], bf, tag="xnm", name="xnm1")
        nc.sync.dma_start(xnm1[:], xnm_d[1])
        for k in range(1, KC):
            t = spool.tile([128, N], bf, tag=f"s{k}", name=f"s{k}")
            nc.sync.dma_start(t[:], sT_d[k])
            s_tiles.append(t)
        binp[0] = [xnm0]
        binp[1] = [xnm1]
        for b in range(B_LOC):
            x0Tb = w2pool.tile([F, N], bf, tag="x0Tb", name=f"x0Tb{b}")
            nc.sync.dma_start(x0Tb[:], x0Tb_d[b])
            hxTf = wpool.tile([UNITS, N], f32, tag="hxTf", name=f"hxTf{b}")
            nc.sync.dma_start(hxTf[:], hxTf_d[b])
            binp[b] += [x0Tb, hxTf]

        def const(name, dram, shape, dt):
            t = cpool.tile(shape, dt, tag=name, name=name)
            nc.sync.dma_start(t[:], dram[:])
            return t

        wru0 = const("wru0", wru0_d, [F, 128], bf)
        wru1 = const("wru1", wru1_d, [F, 128], bf)
        wru2 = const("wru2", wru2_d, [F, 128], bf)
        wcinp = const("wcinp", wcinp_d, [D_IN, UNITS], bf)
        wcrh = const("wcrh", wcrh_d, [UNITS, UNITS], bf)
        wc1 = const("wc1", wc1_d, [F, UNITS], bf)
        wc2 = const("wc2", wc2_d, [F, UNITS], bf)
        ident = const("ident", id_d, [UNITS, UNITS], bf)

        # warm the ACT function table off the critical path (a function-set
        # switch mid-kernel costs ~1.3us)
        dum = cpool.tile([1, 2], f32, tag="dum", name="dum")
        nc.scalar.activation(dum[0:1, 0:1], ident[0:1, 0:1], AF.Sigmoid)
        nc.scalar.activation(dum[0:1, 1:2], ident[0:1, 0:1], AF.Tanh)

        def stream_pass(lhs_xnms, rhs_tiles, dsts, pfx, defer_drain=False):
            # dsts[i] = lhs_xnms[i].T @ rhs_tiles.T, k-major so several
            # batches' matmuls interleave behind one streamed rhs.
            # Drains stay off the ACT engine: an activation-function switch
            # costs a ~1.3us LoadActFuncSet, so ACT runs only sigmoid/tanh.
            nb = len(lhs_xnms)
            ps = [[pp.tile([F, 512], f32, tag="ps", name=f"ps_{pfx}_{i}_{s}")
                   for s in range(NS)] for i in range(nb)]
            for k in range(KC):
                for i in range(nb):
                    lhsT = lhs_xnms[i][:, k, :]
                    for s in range(NS):
                        nc.tensor.matmul(
                            ps[i][s][:], lhsT,
                            rhs_tiles[k][:, s * 512:(s + 1) * 512],
                            start=(k == 0), stop=(k == KC - 1))
            if defer_drain:
                return ps
            for i in range(nb):
                for s in range(NS):
                    nc.vector.tensor_copy(
                        dsts[i][:, s * 512:(s + 1) * 512], ps[i][s][:])

        # ---- gconv 1 S-passes, both batches fused behind one S stream ----
        x1Ts = [w2pool.tile([F, N], bf, tag="x1T", name=f"x1T{b}")
                for b in range(B_LOC)]
        stream_pass([binp[0][0], binp[1][0]], s_tiles, x1Ts, "s1")

        for b in range(B_LOC):
            xnm, x0Tb, hxTf = binp[b]

            a_tiles = []
            for k in range(KC):
                t = apool.tile([128, N], bf, tag="adp", name=f"adp_{b}_{k}")
                nc.sync.dma_start(t[:], adpT_d[b, k])
                a_tiles.append(t)

            # ---- gconv 1 adp-pass ----
            x1T = x1Ts[b]
            x2T = w2pool.tile([F, N], bf, tag="x2T")
            stream_pass([xnm], a_tiles, [x2T], f"a1_{b}")

            # ru = sigmoid(x0.W0 + x1.W1 + x2.W2). r and u are computed as
            # separate accumulation groups (W free-dim split) so both land at
            # partitions 0-63 - two-input DVE ops need equal base partitions.
            # ru = sigmoid([x0|x1|x2] @ W_ru): one M=128 accumulation group
            # per slab; r (rows 0-63) and u (rows 64-127) drain via separate
            # sigmoids, u with a shifted partition base down to 0-63.
            # rh = r*hx follows per slab on the DVE; the PE transposes of rh
            # run after all ru matmuls so their input chain is already done.
            ract = wpool.tile([UNITS, N], f32, tag="ract")
            uact = wpool.tile([UNITS, N], f32, tag="uact")
            rhb = wpool.tile([UNITS, N], bf, tag="rhb")
            ru_ps = []
            for s in range(NS):
                sl = slice(s * 512, (s + 1) * 512)
                ps = pp.tile([128, 512], f32, tag="ps", name=f"ps_ru{s}")
                nc.tensor.matmul(ps[:], wru0[:], x0Tb[:, sl], start=True, stop=False)
                nc.tensor.matmul(ps[:], wru1[:], x1T[:, sl], start=False, stop=False)
                nc.tensor.matmul(ps[:], wru2[:], x2T[:, sl], start=False, stop=True)
                nc.scalar.activation(ract[:, sl], ps[0:UNITS, :], AF.Sigmoid)
                nc.vector.tensor_mul(rhb[:, sl], ract[:, sl], hxTf[:, sl])
                ru_ps.append(ps)
            for s in range(NS):
                # u is needed only at the final combine; keep it off the
                # r -> rh -> transpose critical path
                sl = slice(s * 512, (s + 1) * 512)
                nc.scalar.activation(uact[:, sl], ru_ps[s][UNITS:128, :], AF.Sigmoid)
            for k in range(KC):
                pst = pp.tile([128, 1024], bf, tag="ps", name=f"pst_{k}")
                nc.tensor.transpose(
                    pst[:, 0:UNITS], rhb[:, k * 128:(k + 1) * 128], ident[:])
                nc.vector.tensor_copy(xnm[:, k, D_IN:F], pst[:, 0:UNITS])

            # ---- gconv 2 ----
            x1p = w2pool.tile([F, N], bf, tag="x1T")
            x2p = w2pool.tile([F, N], bf, tag="x2T")
            ps1 = stream_pass([xnm], s_tiles, None, f"s2_{b}", defer_drain=True)
            ps2 = stream_pass([xnm], a_tiles, None, f"a2_{b}", defer_drain=True)
            for s in range(NS):
                dsl = slice(s * 512, (s + 1) * 512)
                nc.vector.tensor_copy(x1p[:, dsl], ps1[0][s][:])
                nc.vector.tensor_copy(x2p[:, dsl], ps2[0][s][:])

            # c = tanh(inp.Wc[0:2] + rh.Wc[2:66] + x1'.Wc1 + x2'.Wc2)
            cT = wpool.tile([UNITS, N], f32, tag="cT")
            outT = wpool.tile([UNITS, N], f32, tag="outT")
            for s in range(NS):
                sl = slice(s * 512, (s + 1) * 512)
                ps = pp.tile([UNITS, 512], f32, tag="ps", name=f"ps_c{s}")
                nc.tensor.matmul(ps[:], wcinp[:], x0Tb[0:D_IN, sl], start=True, stop=False)
                nc.tensor.matmul(ps[:], wcrh[:], rhb[:, sl], start=False, stop=False)
                nc.tensor.matmul(ps[:], wc1[:], x1p[:, sl], start=False, stop=False)
                nc.tensor.matmul(ps[:], wc2[:], x2p[:, sl], start=False, stop=True)
                nc.scalar.activation(cT[:, sl], ps[:], AF.Tanh)
                # out = c + u*(hx - c); alternate slabs between DVE and
                # GpSimd so two dependency chains run in parallel
                eng = nc.vector if s % 2 == 1 else nc.gpsimd
                eng.tensor_sub(outT[:, sl], hxTf[:, sl], cT[:, sl])
                eng.tensor_mul(outT[:, sl], uact[:, sl], outT[:, sl])
                eng.tensor_add(outT[:, sl], outT[:, sl], cT[:, sl])
            nc.sync.dma_start(out_d[b], outT[:])

    nc.compile()
    _CACHE["nc"] = nc
    return nc


def _prep_host(inputs, hx, adp, support_rows, support_cols, support_vals,
               W_ru, W_c):
    xcat = np.concatenate(
        [inputs.reshape(B, N, D_IN), hx.reshape(B, N, UNITS)], axis=2)
    xcat = np.ascontiguousarray(xcat, dtype=np.float32)

    S = np.zeros((N, N), np.float32)
    np.add.at(S, (support_rows, support_cols), support_vals)
    sT = np.ascontiguousarray(S.T).astype(BF16).reshape(KC, 128, N)

    adp_bf = adp.astype(BF16)
    adpT = np.ascontiguousarray(adp_bf.transpose(0, 2, 1)).reshape(B, KC, 128, N)

    xnm = xcat.astype(BF16).reshape(B, KC, 128, F).transpose(0, 2, 1, 3)
    xnm = np.ascontiguousarray(xnm)
    x0T = np.ascontiguousarray(xcat.transpose(0, 2, 1))
    x0Tb = x0T.astype(BF16)
    hxTf = np.ascontiguousarray(x0T[:, D_IN:F])

    wru = {
        "wru0b": np.ascontiguousarray(W_ru[0::3]).astype(BF16),
        "wru1b": np.ascontiguousarray(W_ru[1::3]).astype(BF16),
        "wru2b": np.ascontiguousarray(W_ru[2::3]).astype(BF16),
    }
    wc0 = np.ascontiguousarray(W_c[0::3])
    wcd = {
        "wcinpb": np.ascontiguousarray(wc0[0:D_IN]).astype(BF16),
        "wcrhb": np.ascontiguousarray(wc0[D_IN:F]).astype(BF16),
        "wc1b": np.ascontiguousarray(W_c[1::3]).astype(BF16),
        "wc2b": np.ascontiguousarray(W_c[2::3]).astype(BF16),
    }
    ident = np.eye(UNITS, dtype=BF16)

    shared = {"sT": sT, "ident": ident, **wru, **wcd}
    in_maps = []
    for c in range(N_CORES):
        lo, hi = c * B_LOC, (c + 1) * B_LOC
        in_maps.append({
            "adpT": np.ascontiguousarray(adpT[lo:hi]),
            "xnm": np.ascontiguousarray(xnm[lo:hi]),
            "x0Tb": np.ascontiguousarray(x0Tb[lo:hi]),
            "hxTf": np.ascontiguousarray(hxTf[lo:hi]),
            **shared,
        })
    return in_maps


def kernel(inputs, hx, adp, support_rows, support_cols, support_vals,
           W_ru, W_c, time_axis=None):
    from concourse.bass_utils import run_bass_kernel_spmd

    inputs = np.asarray(inputs, dtype=np.float32)
    hx = np.asarray(hx, dtype=np.float32)
    adp = np.asarray(adp, dtype=np.float32)
    support_rows = np.asarray(support_rows)
    support_cols = np.asarray(support_cols)
    support_vals = np.asarray(support_vals, dtype=np.float32)
    W_ru = np.asarray(W_ru, dtype=np.float32)
    W_c = np.asarray(W_c, dtype=np.float32)

    nc = _build()
    in_maps = _prep_host(inputs, hx, adp, support_rows, support_cols,
                         support_vals, W_ru, W_c)

    res = run_bass_kernel_spmd(nc, in_maps, core_ids=list(range(N_CORES)),
                               trace=False)
    _CACHE["last_result"] = res

    out = np.empty((B, N * UNITS), np.float32)
    for c in range(N_CORES):
        outT = res.results[c]["outT"]  # (B_LOC, 64, N)
        for i in range(B_LOC):
            out[c * B_LOC + i] = np.ascontiguousarray(
                outT[i].T).reshape(N * UNITS)
    return out
